# revision 21
# baseline (speedup 1.0000x reference)
"""Trainium2 Bass kernel for sparse (top-k) attention with memory slots.

Reference computation (per batch b):
  qkv = x @ w_qkv + b_qkv                  -> q,k,v [H, N, HD]
  k,v = concat(k|mem_k), concat(v|mem_v)   -> [H, S=N+M, HD]
  scores = (q @ k^T) * HD**-0.5 * scale[h] ; diag(N,S) masked to -inf
  keep only top-32 scores per row, softmax, @ v, reshape, @ w_proj + b_proj

Sharding: 8 cores = (batch b in 0..3) x (head-half hg in 0..1).
Each core computes, for its 4 heads of its batch, the full attention and
its partial projection sum out_part = sum_h out_h @ w_proj[h].  The host
sums the two half-head partials per batch and adds b_proj.

Design notes (measured on HW):
  * The DVE top-32 scan is the wall: max8 / match_replace / stt run at
    ~1.07 ns/elem (1 elem/cycle) regardless of dtype, so the 7-pass scan
    over S=2052 costs ~16 us per (head, query-tile) and dominates.
  * scores via a 3-term fp16 Dekker split (Qh.Kh + Qh.Kl + Ql.Kh, fp32
    PSUM): fp16 products are exact in fp32, so precision stays in the
    fp32 class (drop term ~3e-8) at ~3x the fp32 matmul rate.  The topk
    selection is rounding-critical (a ~1e-5 score error already breaks
    the 2e-2 gate); bf16 variants are NOT usable.  Host folds a 2^8
    scale into wq/bq/wk/bk/memk so the fp16 low words stay normal; the
    ACT psum->sbuf copy descales by 2^-16 (exact).
  * Q/K fp16 tensors live in head-major [128 = 4h x 32hd, n] layout
    built directly from the projection PSUM; score matmuls address them
    with partition-offset APs.  Matmul operands may only sit at base
    partition 0/32/64, so head 3 uses a replicated copy at partition 0.
  * The 4 heads of a query tile are emitted with their scans
    interleaved, and score production runs under tc.high_priority so
    the Tile scheduler keeps all four scans' inputs ready ahead of the
    DVE (semaphore updates land ~2.2 us after the producing
    instruction, so dependent back-to-back DVE ops stall).

Hardware constraint: a Matmult (incl. transposes) supports only ONE
semaphore wait.  Tile emits one wait per dependency engine, so every
matmul operand tensor (and every PSUM slot's previous reader) must be
produced by the SAME engine.  We use ACT (ScalarE) as the sole producer
of matmul operands: all DMA-landed data is staged through an ACT copy,
identities are host-provided, and P is cast through ACT.
"""

import os
import numpy as np

import concourse.bass as bass
import concourse.mybir as mybir
import concourse.tile as tile
from concourse.alu_op_type import AluOpType

B, N, DIM = 4, 2048, 256
H, HD, M = 8, 32, 4
S = N + M
TOPK = 32
NEG = -1e30
P = 128
HPC = H // 2            # heads per core
NCORES = 8
NT = N // P             # query tiles per (b, h)
NKC = (S + P - 1) // P  # 17 key chunks (last is 4 wide)
QSCALE = 256.0          # 2^8 folded into wq/bq and wk/bk/memk on host
DESCALE = 1.0 / (QSCALE * QSCALE)   # exact 2^-16 applied on psum read

f32 = mybir.dt.float32
bf16 = mybir.dt.bfloat16
f16 = mybir.dt.float16


def build_nc():
    from concourse import bacc
    nc = bacc.Bacc()

    x_d = nc.dram_tensor("x", [N, DIM], f32, kind="ExternalInput")
    wq_d = nc.dram_tensor("wq", [DIM, P], f32, kind="ExternalInput")
    wk_d = nc.dram_tensor("wk", [DIM, P], f32, kind="ExternalInput")
    wv_d = nc.dram_tensor("wv", [DIM, P], f32, kind="ExternalInput")
    bq_d = nc.dram_tensor("bq", [P, 1], f32, kind="ExternalInput")
    bk_d = nc.dram_tensor("bk", [P, 1], f32, kind="ExternalInput")
    bv_d = nc.dram_tensor("bv", [1, P], f32, kind="ExternalInput")
    wp_d = nc.dram_tensor("wp", [HPC, HD, DIM], f32, kind="ExternalInput")
    memk_d = nc.dram_tensor("memk", [P, M], f32, kind="ExternalInput")
    memv_d = nc.dram_tensor("memv", [HPC, M, HD], f32, kind="ExternalInput")
    idf_d = nc.dram_tensor("identf", [P, P], f32, kind="ExternalInput")
    idb_d = nc.dram_tensor("identb", [P, P], bf16, kind="ExternalInput")
    bvb_d = nc.dram_tensor("bvb", [P, P], f32, kind="ExternalInput")
    out_d = nc.dram_tensor("out", [N, DIM], f32, kind="ExternalOutput")

    with tile.TileContext(nc) as tc:
        _body(nc, tc, x_d, wq_d, wk_d, wv_d, bq_d, bk_d, bv_d, wp_d,
              memk_d, memv_d, idf_d, idb_d, bvb_d, out_d)
    nc.finalize()
    return nc


def _body(nc, tc, x_d, wq_d, wk_d, wv_d, bq_d, bk_d, bv_d, wp_d,
          memk_d, memv_d, idf_d, idb_d, bvb_d, out_d):
    Copy = mybir.ActivationFunctionType.Copy
    Ident = mybir.ActivationFunctionType.Identity
    Exp = mybir.ActivationFunctionType.Exp

    import contextlib
    stack = contextlib.ExitStack()
    with stack:
        const = stack.enter_context(tc.tile_pool(name="const", bufs=1))
        persist = stack.enter_context(tc.tile_pool(name="persist", bufs=1))

        # persistent per-core tensors.  All matmul operands => ACT-written.
        Qh = persist.tile([P, N], f16)
        Ql = persist.tile([P, N], f16)
        Kh = persist.tile([P, S], f16)
        Kl = persist.tile([P, S], f16)
        # matmul operand base partition must be 0/32/64: head 3 (offset 96)
        # gets a replicated copy at partition 0
        Qh3 = persist.tile([HD, N], f16)
        Ql3 = persist.tile([HD, N], f16)
        Kh3 = persist.tile([HD, S], f16)
        Kl3 = persist.tile([HD, S], f16)
        V = persist.tile([P, NKC, P], bf16)             # [s%128, chunk, 4h*32hd]
        wp_sb = persist.tile([HD, HPC, DIM], f32)
        ident_fp = const.tile([P, P], f32)   # DVE-written: prologue matmuls
        ident_bp = const.tile([P, P], bf16)  # DVE-written: warming dummies
        ident_f = const.tile([P, P], f32)    # ACT-written: main-loop matmuls
        ident_b = const.tile([P, P], bf16)   # ACT-written
        bvb_sb = const.tile([P, P], f32)
        w_sb = {}
        for name in ("q", "k", "v"):
            w_sb[name] = const.tile([P, DIM // P, P], f32, name=f"w_{name}")

        # biases for the DVE psum-combine (per-partition scalar ptrs)
        bq_sb = const.tile([P, 1], f32)
        bk_sb = const.tile([P, 1], f32)
        nc.sync.dma_start(bq_sb[:], bq_d[:])
        nc.sync.dma_start(bk_sb[:], bk_d[:])

        with tc.tile_pool(name="pro_sb", bufs=1) as pro_sb, \
             tc.tile_pool(name="pro_work", bufs=4) as pro_work, \
             tc.tile_pool(name="pro_ps", bufs=2, space="PSUM") as pro_ps, \
             tc.tile_pool(name="pro_ps_qk", bufs=2, space="PSUM") as pro_ps_qk, \
             tc.tile_pool(name="pro_ps_v", bufs=1, space="PSUM") as pro_ps_v:

            # Prologue matmuls (xT transposes, qkv projections) read
            # DVE-written tiles and their PSUM is read back by DVE;
            # main-loop matmuls read ACT-written tiles and their PSUM is
            # read back by ACT (one-wait rule, see module docstring).
            def staged_via(eng_copy, dst_ap, dram_ap, shape, dtype, tag):
                raw = pro_sb.tile(shape, dtype, tag=tag, name=f"raw_{tag}")
                nc.sync.dma_start(raw[:], dram_ap)
                eng_copy(dst_ap, raw[:])

            def staged_dve(dst_ap, dram_ap, shape, dtype, tag):
                staged_via(nc.vector.tensor_copy, dst_ap, dram_ap,
                           shape, dtype, tag)

            def staged_act(dst_ap, dram_ap, shape, dtype, tag):
                staged_via(nc.scalar.copy, dst_ap, dram_ap, shape, dtype, tag)

            staged_dve(ident_fp[:], idf_d[:], [P, P], f32, "idfp")
            staged_dve(ident_bp[:], idb_d[:], [P, P], bf16, "idbp")
            staged_act(ident_f[:], idf_d[:], [P, P], f32, "idf")
            staged_act(ident_b[:], idb_d[:], [P, P], bf16, "idb")
            # bvb (v-bias broadcast) feeds only DVE adds, no staging needed
            nc.sync.dma_start(bvb_sb[:], bvb_d[:])
            staged_act(wp_sb[:], wp_d.rearrange("h p c -> p h c"),
                       [HD, HPC, DIM], f32, "wp")
            for name, wd in (("q", wq_d), ("k", wk_d), ("v", wv_d)):
                staged_dve(w_sb[name][:],
                           wd.rearrange("(ko p) c -> p ko c", p=P),
                           [P, DIM // P, P], f32, f"w{name}")

            # ---- x^T ----
            # single DMA + single DVE staging copy for all of x
            x_stage = pro_sb.tile([P, NT, DIM], f32, tag="xstage")
            xT = pro_sb.tile([P, DIM // P, N], f32, tag="xT")
            staged_dve(x_stage[:], x_d.rearrange("(nt p) d -> p nt d", p=P),
                       [P, NT, DIM], f32, "xraw")
            for nt in range(NT):
                r0 = nt * P
                for ko in range(DIM // P):
                    ps = pro_ps.tile([P, P], f32, tag="xtp")
                    nc.tensor.transpose(ps[:],
                                        x_stage[:, nt, ko * P:(ko + 1) * P],
                                        ident_fp[:])
                    nc.vector.tensor_copy(xT[:, ko, r0:r0 + P], ps[:])

            # ---- Q/K: pre = w^T @ x^T + bias (fp32), then fp16 split ----
            # Single-instruction PSUM groups only in the prologue (see v1).
            for dsth, dstl, dsth3, dstl3, wname, bias in (
                    (Qh, Ql, Qh3, Ql3, "q", bq_sb),
                    (Kh, Kl, Kh3, Kl3, "k", bk_sb)):
                for c4 in range(N // 512):
                    cs = slice(c4 * 512, (c4 + 1) * 512)
                    psa = pro_ps_qk.tile([P, 512], f32, tag="qkpa")
                    psb = pro_ps_qk.tile([P, 512], f32, tag="qkpb")
                    nc.tensor.matmul(psa[:], lhsT=w_sb[wname][:, 0, :],
                                     rhs=xT[:, 0, cs], start=True, stop=True)
                    nc.tensor.matmul(psb[:], lhsT=w_sb[wname][:, 1, :],
                                     rhs=xT[:, 1, cs], start=True, stop=True)
                    pre = pro_work.tile([P, 512], f32, tag="qkpre")
                    nc.vector.tensor_copy(pre[:], psa[:])
                    nc.vector.scalar_tensor_tensor(
                        out=pre[:], in0=pre[:], scalar=bias[:], in1=psb[:],
                        op0=AluOpType.add, op1=AluOpType.add)
                    # hi = fp16(pre); lo = fp16(pre - hi)  (Dekker split)
                    nc.scalar.copy(dsth[:, cs], pre[:])
                    low = pro_work.tile([P, 512], f32, tag="qklow")
                    nc.vector.tensor_tensor(out=low[:], in0=pre[:],
                                            in1=dsth[:, cs],
                                            op=AluOpType.subtract)
                    nc.scalar.copy(dstl[:, cs], low[:])
                    # head-3 replica at base partition 0
                    h3 = slice(3 * HD, 4 * HD)
                    nc.scalar.copy(dsth3[:, cs], pre[h3, :])
                    nc.scalar.copy(dstl3[:, cs], low[h3, :])

            # mem_k columns (host pre-scaled/arranged [128, M])
            mk_raw = pro_work.tile([P, M], f32, tag="mkraw")
            nc.sync.dma_start(mk_raw[:], memk_d[:])
            mk_stage = pro_work.tile([P, M], f32, tag="mks")
            nc.vector.tensor_copy(mk_stage[:], mk_raw[:])
            nc.scalar.copy(Kh[:, N:S], mk_stage[:])
            mk_low = pro_work.tile([P, M], f32, tag="mklow")
            nc.vector.tensor_tensor(out=mk_low[:], in0=mk_stage[:],
                                    in1=Kh[:, N:S], op=AluOpType.subtract)
            nc.scalar.copy(Kl[:, N:S], mk_low[:])
            nc.scalar.copy(Kh3[:, N:S], mk_stage[3 * HD:4 * HD, :])
            nc.scalar.copy(Kl3[:, N:S], mk_low[3 * HD:4 * HD, :])

            # ---- V = x @ w_v + bv  (bf16) ----
            for nt in range(NT):
                r0 = nt * P
                psa = pro_ps_v.tile([P, P], f32, tag="vpa")
                psb = pro_ps_v.tile([P, P], f32, tag="vpb")
                nc.tensor.matmul(psa[:], lhsT=xT[:, 0, r0:r0 + P],
                                 rhs=w_sb["v"][:, 0, :], start=True, stop=True)
                nc.tensor.matmul(psb[:], lhsT=xT[:, 1, r0:r0 + P],
                                 rhs=w_sb["v"][:, 1, :], start=True, stop=True)
                vpre = pro_work.tile([P, P], f32, tag="vpre")
                nc.vector.tensor_tensor(out=vpre[:], in0=psa[:], in1=bvb_sb[:],
                                        op=AluOpType.add)
                nc.vector.tensor_tensor(out=vpre[:], in0=vpre[:], in1=psb[:],
                                        op=AluOpType.add)
                nc.scalar.copy(V[:, nt, :], vpre[:])
            mv_stage = pro_work.tile([M, HPC * HD], f32, tag="mvs")
            for h in range(HPC):
                nc.sync.dma_start(mv_stage[:, HD * h:HD * (h + 1)], memv_d[h])
            nc.scalar.copy(V[0:M, NT, :], mv_stage[:])

        # ---------------- main loop ----------------
        sb_scores = stack.enter_context(tc.tile_pool(name="scores", bufs=5))
        sb_scratch = stack.enter_context(tc.tile_pool(name="scratch", bufs=4))
        sb_e = stack.enter_context(tc.tile_pool(name="esb", bufs=4))
        sb_p = stack.enter_context(tc.tile_pool(name="psb", bufs=2))
        sb_small = stack.enter_context(tc.tile_pool(name="small", bufs=6))
        sb_pt = stack.enter_context(tc.tile_pool(name="ptsb", bufs=2))
        sb_out = stack.enter_context(tc.tile_pool(name="outsb", bufs=2))

        ps_scores = stack.enter_context(
            tc.tile_pool(name="ps_sc", bufs=2, space="PSUM"))
        ps_pt = stack.enter_context(
            tc.tile_pool(name="ps_pt", bufs=2, space="PSUM"))
        ps_tail = stack.enter_context(
            tc.tile_pool(name="ps_tail", bufs=1, space="PSUM"))
        ps_av = stack.enter_context(
            tc.tile_pool(name="ps_av", bufs=1, space="PSUM"))
        ps_ot = stack.enter_context(
            tc.tile_pool(name="ps_ot", bufs=1, space="PSUM"))
        ps_proj = stack.enter_context(
            tc.tile_pool(name="ps_proj", bufs=1, space="PSUM"))

        CH = [(0, 512), (512, 512), (1024, 512), (1536, 512), (2048, M)]

        # one shared fill register: affine_select allocates a fresh Pool
        # register per float fill, and the Pool engine only has ~48.
        neg_reg = nc.gpsimd.to_reg(NEG)

        # Phase fence for the PE engine (see v1 notes): absorb prologue
        # bank releases / unobserved prologue ticks so main-loop matmuls
        # only ever carry their single fresh ACT wait.
        from concourse.tile import add_dep_helper
        cur_bb = nc.cur_bb
        assert cur_bb is not None
        prologue_insts = list(cur_bb.bb.instructions)

        def dep_on(obs, engine):
            for inst in prologue_insts:
                if inst.engine == engine:
                    add_dep_helper(
                        obs.ins, inst,
                        sync=bass.sync_unless_reorderable_target(
                            inst, inst.is_executable()),
                        reason="pe observes prologue")

        fs0 = ps_scores.tile([P, 512], f32, tag="sc", name="fs0")
        m_a = nc.tensor.matmul(fs0[:, 0:P], lhsT=ident_fp[:], rhs=ident_fp[:],
                               start=True, stop=False)
        m_b = nc.tensor.matmul(fs0[:, 0:P], lhsT=ident_fp[:], rhs=ident_fp[:],
                               start=False, stop=False)
        dep_on(m_b, mybir.EngineType.DVE)
        m_c = nc.tensor.matmul(fs0[:, 0:P], lhsT=ident_f[:], rhs=ident_f[:],
                               start=False, stop=True)
        dep_on(m_c, mybir.EngineType.Activation)
        # order the whole main loop after the observers
        tc.no_sync_barrier()

        for nt in range(NT):
            r0 = nt * P
            outT = sb_out.tile([HD, HPC, P], f32, tag="outT")
            s_t, scr_t, e_t, vals_t = {}, {}, {}, {}
            sums_t, recip_t, praw_t = {}, {}, {}

            # ---- scores for all 4 heads (PE/ACT/Pool) ----
            # high_priority: front-load score production so all 4 heads'
            # scans are ready together -- the scheduler then interleaves
            # the scans by emission priority, keeping every DVE wait >= 2
            # instructions back (just-satisfied immediate-predecessor
            # waits cost ~1.2-2.5us of sem-observation latency each).
            tc_hp = tc.high_priority(offset=800)
            tc_hp.__enter__()
            for X in range(HPC):
                h0 = HD * X
                hs = slice(h0, h0 + HD)
                if X < 3:
                    qh, ql = Qh[hs, r0:r0 + P], Ql[hs, r0:r0 + P]
                    kh_t, kl_t = Kh, Kl
                    khs = kls = hs
                else:
                    qh, ql = Qh3[:, r0:r0 + P], Ql3[:, r0:r0 + P]
                    kh_t, kl_t = Kh3, Kl3
                    khs = kls = slice(0, HD)
                s_sb = sb_scores.tile([P, S], f32, tag="s", name=f"s_{nt}_{X}")
                s_t[X] = s_sb
                for c0, cw in CH:
                    csl = slice(c0, c0 + cw)
                    psc = ps_scores.tile([P, 512], f32, tag="sc")
                    nc.tensor.matmul(psc[:, :cw], lhsT=qh,
                                     rhs=kh_t[khs, csl], start=True, stop=False)
                    nc.tensor.matmul(psc[:, :cw], lhsT=qh,
                                     rhs=kl_t[kls, csl], start=False, stop=False)
                    nc.tensor.matmul(psc[:, :cw], lhsT=ql,
                                     rhs=kh_t[khs, csl], start=False, stop=True)
                    nc.scalar.activation(out=s_sb[:, csl], in_=psc[:, :cw],
                                         func=Copy, scale=DESCALE)
                # diagonal mask: element (p, r0+p) -> NEG
                nc.gpsimd.affine_select(
                    out=s_sb[:, r0:r0 + P], in_=s_sb[:, r0:r0 + P],
                    compare_op=mybir.AluOpType.not_equal,
                    fill=neg_reg, base=0, pattern=[[-1, P]],
                    channel_multiplier=1)
                # E = exp(s): independent of the scan, issue early on ACT
                e_sb = sb_e.tile([P, S], f32, tag="e", name=f"e_{nt}_{X}")
                e_t[X] = e_sb
                nc.scalar.activation(out=e_sb[:], in_=s_sb[:], func=Exp)
            tc_hp.__exit__(None, None, None)

            # ---- top-32 scans, 4-way head-interleaved on DVE ----
            for X in range(HPC):
                vals_t[X] = sb_small.tile([P, TOPK], f32, tag="vals",
                                          name=f"vals_{nt}_{X}")
                scr_t[X] = sb_scratch.tile([P, S], f32, tag="scr",
                                           name=f"scr_{nt}_{X}")
            def stt(X):
                # write the masked numerators over scr (dead after round 4)
                praw_t[X] = scr_t[X]
                sums_t[X] = sb_small.tile([P, 1], f32, tag="sums",
                                          name=f"sums_{nt}_{X}")
                nc.vector.scalar_tensor_tensor(
                    out=praw_t[X][:], in0=s_t[X][:],
                    scalar=vals_t[X][:, TOPK - 1:TOPK], in1=e_t[X][:],
                    op0=AluOpType.is_ge, op1=AluOpType.mult,
                    accum_out=sums_t[X][:])

            for rnd in range(4):
                for X in range(HPC):
                    src = s_t[X] if rnd == 0 else scr_t[X]
                    nc.vector.max(
                        out=vals_t[X][:, 8 * rnd:8 * rnd + 8], in_=src[:])
                    # P = (s >= thr) * E (+ row sums): chase round 4 so the
                    # stt's vals dep is always >= 2 DVE ops back
                    if rnd == 3 and X >= 2:
                        stt(X - 2)
                if rnd < 3:
                    for X in range(HPC):
                        src = s_t[X] if rnd == 0 else scr_t[X]
                        nc.vector.match_replace(
                            out=scr_t[X][:],
                            in_to_replace=vals_t[X][:, 8 * rnd:8 * rnd + 8],
                            in_values=src[:], imm_value=NEG)
            stt(2)
            stt(3)
            for X in range(HPC):
                recip_t[X] = sb_small.tile([P, 1], f32, tag="recip",
                                           name=f"recip_{nt}_{X}")
                nc.vector.reciprocal(recip_t[X][:], sums_t[X][:])

            # ---- value path per head ----
            for X in range(HPC):
                h0 = HD * X
                # cast to bf16 on ACT (sole matmul-feed engine)
                p_sb = sb_p.tile([P, S], bf16, tag="p", name=f"p_{nt}_{X}")
                nc.scalar.copy(p_sb[:], praw_t[X][:])

                av = ps_av.tile([P, HD], f32, tag="av")
                for g in range(4):   # groups of 4 chunks -> one [128,512] psum
                    ptp = ps_pt.tile([P, 512], bf16, tag="pt")
                    for j in range(4):
                        c = 4 * g + j
                        nc.tensor.transpose(ptp[:, j * P:(j + 1) * P],
                                            p_sb[:, c * P:(c + 1) * P],
                                            ident_b[:])
                    pts = sb_pt.tile([P, 512], bf16, tag="pts")
                    nc.scalar.copy(pts[:], ptp[:])
                    for j in range(4):
                        c = 4 * g + j
                        nc.tensor.matmul(av[:], lhsT=pts[:, j * P:(j + 1) * P],
                                         rhs=V[:, c, h0:h0 + HD],
                                         start=(c == 0), stop=False)
                # tail chunk (mem slots, 4 wide)
                ptt = ps_tail.tile([M, P], bf16, tag="ptt")
                nc.tensor.transpose(ptt[:], p_sb[:, N:S], ident_b[:])
                ptts = sb_pt.tile([M, P], bf16, tag="ptts")
                nc.scalar.copy(ptts[:], ptt[:])
                nc.tensor.matmul(av[:], lhsT=ptts[:],
                                 rhs=V[0:M, NT, h0:h0 + HD],
                                 start=False, stop=True)

                # ---- out tile (normalize by 1/rowsum here) + transpose ----
                outn = sb_out.tile([P, HD], f32, tag="outn")
                nc.scalar.activation(out=outn[:], in_=av[:], func=Copy,
                                     scale=recip_t[X][:])
                pso = ps_ot.tile([HD, P], f32, tag="ot")
                nc.tensor.transpose(pso[:], outn[:], ident_f[:])
                nc.scalar.copy(outT[:, X, :], pso[:])

            # ---- projection: sum_h outT_h.T @ wp_h ----
            psp = ps_proj.tile([P, DIM], f32, tag="proj")
            for X in range(HPC):
                nc.tensor.matmul(psp[:], lhsT=outT[:, X, :],
                                 rhs=wp_sb[:, X, :],
                                 start=(X == 0), stop=(X == HPC - 1))
            prj = sb_out.tile([P, DIM], f32, tag="prj")
            nc.scalar.copy(prj[:], psp[:])
            nc.sync.dma_start(out_d[r0:r0 + P, :], prj[:])


_NC_CACHE = None


def _get_nc():
    global _NC_CACHE
    if _NC_CACHE is None:
        _NC_CACHE = build_nc()
    return _NC_CACHE


def make_in_maps(inputs):
    import ml_dtypes

    x = np.asarray(inputs["x"], dtype=np.float32)
    w_qkv = np.asarray(inputs["w_qkv"], dtype=np.float32)
    b_qkv = np.asarray(inputs["b_qkv"], dtype=np.float32)
    w_proj = np.asarray(inputs["w_proj"], dtype=np.float32)
    scale = np.asarray(inputs["scale"], dtype=np.float32).reshape(H)
    mem_k = np.asarray(inputs["mem_k"], dtype=np.float32)
    mem_v = np.asarray(inputs["mem_v"], dtype=np.float32)

    w4 = w_qkv.reshape(DIM, 3, H, HD)
    b3 = b_qkv.reshape(3, H, HD)
    wp3 = w_proj.reshape(H, HD, DIM)
    identf = np.eye(P, dtype=np.float32)
    identb = np.eye(P).astype(ml_dtypes.bfloat16)

    in_maps = []
    for c in range(NCORES):
        b, hg = c // 2, c % 2
        hs = slice(hg * HPC, (hg + 1) * HPC)
        sc = (scale[hs] * HD ** -0.5 * QSCALE).astype(np.float32)   # [HPC]
        in_maps.append({
            "x": np.ascontiguousarray(x[b]),
            "wq": np.ascontiguousarray(
                (w4[:, 0, hs] * sc[None, :, None]).reshape(DIM, P)),
            "wk": np.ascontiguousarray(
                (w4[:, 1, hs] * QSCALE).reshape(DIM, P)),
            "wv": np.ascontiguousarray(w4[:, 2, hs].reshape(DIM, P)),
            "bq": np.ascontiguousarray(
                (b3[0, hs] * sc[:, None]).reshape(P, 1)),
            "bk": np.ascontiguousarray(
                (b3[1, hs] * QSCALE).reshape(P, 1)),
            "bv": np.ascontiguousarray(b3[2, hs].reshape(1, P)),
            "wp": np.ascontiguousarray(wp3[hs]),
            # memk: [4h x 32hd, M] pre-scaled by 2^8
            "memk": np.ascontiguousarray(
                (mem_k[hs] * QSCALE).transpose(0, 2, 1).reshape(P, M)),
            "memv": np.ascontiguousarray(mem_v[hs]),
            "identf": identf,
            "identb": identb,
            "bvb": np.ascontiguousarray(
                np.broadcast_to(b3[2, hs].reshape(1, P), (P, P))),
        })
    return in_maps


def gather(results, b_proj):
    outs = [np.asarray(r["out"], dtype=np.float32) for r in results]
    full = np.stack([outs[2 * b] + outs[2 * b + 1] for b in range(B)])
    return (full + np.asarray(b_proj, dtype=np.float32)).astype(np.float32)


def run(inputs, **kwargs):
    from concourse.bass_utils import run_bass_kernel_spmd
    nc = _get_nc()
    in_maps = make_in_maps(inputs)
    res = run_bass_kernel_spmd(nc, in_maps, core_ids=list(range(NCORES)),
                               **kwargs)
    return gather(res.results, inputs["b_proj"]), res


def kernel(**inputs):
    out, _ = run(inputs)
    return out
